# revision 8
# baseline (speedup 1.0000x reference)
"""Multi-head self-attention on 8 Trainium2 NeuronCores.

Sharding: data-parallel over batch (B=8 -> one batch element per core).
Each core runs the full MHA for its batch element; no collectives.

Per-core layout strategy (all matmuls in float32r = full-rate fp32):
  - host supplies xT = x.T [E,S] (e-major) and w_inT = w_in.T [E,3E].
  - in_proj Q/K computed transposed: projT[f,s] (f-major) -> per-head
    Q_hT/K_hT [d=64, s] slices are directly the matmul operands for
    scoresT[k,q] = K_hT.T @ Q_hT  (softmax scale 1/8 and q-bias folded
    into w_inT's Q columns / bq8 on host; k/v biases fold out of softmax
    or into the output bias).
  - in_proj V computed untransposed: proj_v[s, f] (s=k-major), stored
    head-strided with a ones column per head -> AV matmul
    lhsT=[V_h|1][k,65], rhs=attnT[k,q] yields out rows 0:64 = V^T E and
    row 64 = softmax denominator per q.
  - normalize: recip(denom row) -> partition_broadcast -> fused multiply
    on the PSUM->SBUF copyback into attnoutT[f,s].
  - out_proj transposed: finalT[e,s] = w_outT.T @ attnoutT (+ b_out_eff
    per-partition); host transposes back.
"""

import sys

if "/opt/trn_rl_repo" not in sys.path:
    sys.path.insert(0, "/opt/trn_rl_repo")

import numpy as np

B, S, E, H, D = 8, 1024, 1024, 16, 64
P = 128
EC = E // P  # 8 e-chunks
SB = S // 512  # 2 s-blocks

_NC = None
DEBUG_DUMPS = False


def attention_body(
    nc, tc, Act, f32, f32r, projq, projk, projv, attnout, attn_p, dnm_p, bca_p, ps_s, ps_av, dbg=None
):
    for qb in range(SB):
        for hp in range(EC):  # head pair = f-chunk of projq/projk
            at_tiles = [[None] * EC for _ in range(2)]
            for kc in range(EC):
                for half in range(2):
                    ps = ps_s.tile([P, 512], f32, name="ps_sc")
                    r0, r1 = half * 64, (half + 1) * 64
                    nc.tensor.matmul(
                        ps[:, :],
                        projk[hp][r0:r1, kc * 128 : (kc + 1) * 128],
                        projq[hp][r0:r1, qb * 512 : (qb + 1) * 512],
                        start=True,
                        stop=True,
                        tile_position=(64 * half, 0),
                    )
                    at = attn_p.tile([P, 512], f32r, name="at")
                    nc.scalar.activation(at[:, :], ps[:, :], Act.Exp)
                    at_tiles[half][kc] = at
                    if dbg is not None and DEBUG_DUMPS and qb == 0 and hp == 0 and kc == 0 and half == 0:
                        nc.sync.dma_start(out=dbg["at00"][:, :], in_=at[:, :])
            for half in range(2):
                h = hp * 2 + half
                pv = ps_av.tile([65, 512], f32, name="pv")
                for kc in range(EC):
                    nc.tensor.matmul(
                        pv[:, :],
                        projv[kc][:, h * 65 : h * 65 + 65],
                        at_tiles[half][kc][:, :],
                        start=(kc == 0),
                        stop=(kc == EC - 1),
                    )
                dr = dnm_p.tile([1, 512], f32, name="dr")
                nc.vector.tensor_copy(dr[:, :], pv[64:65, :])
                rc = dnm_p.tile([1, 512], f32, name="rc")
                nc.vector.reciprocal_approx_fast(rc[:, :], dr[:, :])
                bc = bca_p.tile([64, 512], f32, name="bc")
                nc.gpsimd.partition_broadcast(bc[:, :], rc[:, :])
                if dbg is not None and DEBUG_DUMPS and qb == 0 and hp == 0 and half == 0:
                    nc.sync.dma_start(out=dbg["bc0"][:, :], in_=bc[:, :])
                nc.vector.tensor_mul(
                    attnout[hp][
                        half * 64 : (half + 1) * 64,
                        qb * 512 : (qb + 1) * 512,
                    ],
                    pv[0:64, :],
                    bc[:, :],
                )


def build_nc():
    global _NC
    if _NC is not None:
        return _NC

    import concourse.mybir as mybir
    from concourse import bacc
    from concourse.tile import TileContext

    f32 = mybir.dt.float32
    f32r = mybir.dt.float32r
    Act = mybir.ActivationFunctionType

    nc = bacc.Bacc("TRN2", target_bir_lowering=False, debug=False, num_devices=8)

    xT_d = nc.dram_tensor("xT", [E, S], f32r, kind="ExternalInput").ap()
    w_inT_d = nc.dram_tensor("w_inT", [E, 3 * E], f32r, kind="ExternalInput").ap()
    bq8_d = nc.dram_tensor("bq8", [E, 1], f32, kind="ExternalInput").ap()
    w_outT_d = nc.dram_tensor("w_outT", [E, E], f32r, kind="ExternalInput").ap()
    bo_d = nc.dram_tensor("bo", [E, 1], f32, kind="ExternalInput").ap()
    outT_d = nc.dram_tensor("outT", [E, S], f32, kind="ExternalOutput").ap()
    dbg = {}
    if DEBUG_DUMPS:
        dbg["projq0"] = nc.dram_tensor("d_projq0", [P, S], f32r, kind="ExternalOutput").ap()
        dbg["projk0"] = nc.dram_tensor("d_projk0", [P, S], f32r, kind="ExternalOutput").ap()
        dbg["projv0"] = nc.dram_tensor("d_projv0", [P, H * 65], f32r, kind="ExternalOutput").ap()
        dbg["at00"] = nc.dram_tensor("d_at00", [P, 512], f32r, kind="ExternalOutput").ap()
        dbg["attnout0"] = nc.dram_tensor("d_attnout0", [P, S], f32r, kind="ExternalOutput").ap()
        dbg["bc0"] = nc.dram_tensor("d_bc0", [64, 512], f32, kind="ExternalOutput").ap()

    with TileContext(nc) as tc:
        with (
            tc.tile_pool(name="const", bufs=1) as const,
            tc.tile_pool(name="projq", bufs=1) as projq_p,
            tc.tile_pool(name="projk", bufs=1) as projk_p,
            tc.tile_pool(name="projv", bufs=1) as projv_p,
        ):
            bq8_t = []
            bo_t = []
            for i in range(EC):
                t = const.tile([P, 1], f32, tag=f"bq{i}", name=f"bq{i}")
                nc.sync.dma_start(out=t[:, :], in_=bq8_d[i * P : (i + 1) * P, :])
                bq8_t.append(t)
                t = const.tile([P, 1], f32, tag=f"bo{i}", name=f"bo{i}")
                nc.sync.dma_start(out=t[:, :], in_=bo_d[i * P : (i + 1) * P, :])
                bo_t.append(t)

            projq = [projq_p.tile([P, S], f32r, tag=f"pq{i}", name=f"pq{i}") for i in range(EC)]
            projk = [projk_p.tile([P, S], f32r, tag=f"pk{i}", name=f"pk{i}") for i in range(EC)]
            # per s-chunk: 16 heads x (64 V cols + ones col)
            projv = [projv_p.tile([P, H * 65], f32r, tag=f"pv{i}", name=f"pv{i}") for i in range(EC)]

            # ---------------- Phase A: in_proj ----------------
            with (
                tc.tile_pool(name="xT", bufs=1) as xT_p,
                tc.tile_pool(name="w_in", bufs=16) as w_in_p,
                tc.tile_pool(name="ps_a", bufs=6, space="PSUM") as ps_a,
            ):
                xT = []
                for e in range(EC):
                    t = xT_p.tile([P, S], f32r, tag=f"x{e}", name=f"x{e}")
                    nc.sync.dma_start(out=t[:, :], in_=xT_d[e * P : (e + 1) * P, :])
                    xT.append(t)

                # ones columns of projv (col 64 of each head's 65-col group)
                ones_f32 = const.tile([P, H], f32, name="ones_f32")
                nc.vector.memset(ones_f32[:, :], 1.0)
                ones_src = ones_f32[:, :].rearrange("p (h o) -> p h o", o=1)
                for sc in range(EC):
                    ones_ap = projv[sc].rearrange("p (h x) -> p h x", x=65)[:, :, 64:65]
                    nc.vector.tensor_copy(ones_ap, ones_src)

                for fg in range(6):  # f-groups of 512 cols: 0,1=Q 2,3=K 4,5=V
                    w_t = []
                    for e in range(EC):
                        t = w_in_p.tile([P, 512], f32r, name="w_t")
                        nc.sync.dma_start(
                            out=t[:, :],
                            in_=w_inT_d[e * P : (e + 1) * P, fg * 512 : (fg + 1) * 512],
                        )
                        w_t.append(t)
                    if fg < 4:  # Q/K: projT[f,s] orientation
                        for fc in range(4):
                            fglob = fg * 4 + fc  # global f-chunk 0..15
                            for sb in range(SB):
                                ps = ps_a.tile([P, 512], f32)
                                for e in range(EC):
                                    nc.tensor.matmul(
                                        ps[:, :],
                                        w_t[e][:, fc * 128 : (fc + 1) * 128],
                                        xT[e][:, sb * 512 : (sb + 1) * 512],
                                        start=(e == 0),
                                        stop=(e == EC - 1),
                                    )
                                if fg < 2:  # Q: add folded bias
                                    nc.vector.tensor_scalar_add(
                                        projq[fglob][:, sb * 512 : (sb + 1) * 512],
                                        ps[:, :],
                                        bq8_t[fglob][:, 0:1],
                                    )
                                else:  # K: plain copy
                                    nc.vector.tensor_copy(
                                        projk[fglob - 8][:, sb * 512 : (sb + 1) * 512],
                                        ps[:, :],
                                    )
                    else:  # V: proj_v[s, f] orientation
                        h0 = (fg - 4) * 8  # first head in this 512-col group
                        for sc in range(EC):
                            ps = ps_a.tile([P, 512], f32)
                            for e in range(EC):
                                nc.tensor.matmul(
                                    ps[:, :],
                                    xT[e][:, sc * 128 : (sc + 1) * 128],
                                    w_t[e][:, :],
                                    start=(e == 0),
                                    stop=(e == EC - 1),
                                )
                            out_ap = projv[sc].rearrange("p (h x) -> p h x", x=65)[
                                :, h0 : h0 + 8, 0:64
                            ]
                            in_ap = ps.rearrange("p (h d) -> p h d", d=64)
                            nc.vector.tensor_copy(out_ap, in_ap)

            if DEBUG_DUMPS:
                nc.sync.dma_start(out=dbg["projq0"][:, :], in_=projq[0][:, :])
                nc.sync.dma_start(out=dbg["projk0"][:, :], in_=projk[0][:, :])
                nc.sync.dma_start(out=dbg["projv0"][:, :], in_=projv[0][:, :])

            # ---------------- Phase B: attention ----------------
            with tc.tile_pool(name="attnout", bufs=1) as attnout_p:
                attnout = [
                    attnout_p.tile([P, S], f32r, tag=f"ao{i}", name=f"ao{i}") for i in range(EC)
                ]
                with (
                    tc.tile_pool(name="attn", bufs=18) as attn_p,
                    tc.tile_pool(name="dnm", bufs=4) as dnm_p,
                    tc.tile_pool(name="bca", bufs=4) as bca_p,
                    tc.tile_pool(name="ps_s", bufs=6, space="PSUM") as ps_s,
                    tc.tile_pool(name="ps_av", bufs=2, space="PSUM") as ps_av,
                ):
                    attention_body(nc, tc, Act, f32, f32r, projq, projk, projv, attnout, attn_p, dnm_p, bca_p, ps_s, ps_av, dbg)

                if DEBUG_DUMPS:
                    nc.sync.dma_start(out=dbg["attnout0"][:, :], in_=attnout[0][:, :])

                # ---------------- Phase C: out_proj ----------------
                with (
                    tc.tile_pool(name="w_out", bufs=1) as w_out_p,
                    tc.tile_pool(name="fin", bufs=4) as fin_p,
                    tc.tile_pool(name="ps_o", bufs=4, space="PSUM") as ps_o,
                ):
                    w_out_t = []
                    for fc in range(EC):
                        t = w_out_p.tile([P, E], f32r, tag=f"wo{fc}", name=f"wo{fc}")
                        nc.sync.dma_start(
                            out=t[:, :], in_=w_outT_d[fc * P : (fc + 1) * P, :]
                        )
                        w_out_t.append(t)
                    for qb in range(SB):
                        for ec in range(EC):
                            ps = ps_o.tile([P, 512], f32)
                            for fc in range(EC):
                                nc.tensor.matmul(
                                    ps[:, :],
                                    w_out_t[fc][:, ec * 128 : (ec + 1) * 128],
                                    attnout[fc][:, qb * 512 : (qb + 1) * 512],
                                    start=(fc == 0),
                                    stop=(fc == EC - 1),
                                )
                            fin = fin_p.tile([P, 512], f32)
                            nc.vector.tensor_scalar_add(
                                fin[:, :], ps[:, :], bo_t[ec][:, 0:1]
                            )
                            nc.sync.dma_start(
                                out=outT_d[
                                    ec * P : (ec + 1) * P, qb * 512 : (qb + 1) * 512
                                ],
                                in_=fin[:, :],
                            )

    nc.compile()
    _NC = nc
    return nc


def prepare_in_maps(qkv, w_in, b_in, w_out, b_out):
    qkv = np.asarray(qkv, dtype=np.float32)
    w_in = np.asarray(w_in, dtype=np.float32)
    b_in = np.asarray(b_in, dtype=np.float32)
    w_out = np.asarray(w_out, dtype=np.float32)
    b_out = np.asarray(b_out, dtype=np.float32)

    w_inT = np.ascontiguousarray(w_in.T).copy()
    w_inT[:, :E] *= 0.125  # fold softmax scale into Q columns
    bq8 = (b_in[:E] * 0.125).astype(np.float32).reshape(E, 1)
    # v-bias folds through attention (rows of softmax sum to 1) into out bias
    bo_eff = (b_out + w_out @ b_in[2 * E :]).astype(np.float32).reshape(E, 1)
    w_outT = np.ascontiguousarray(w_out.T)

    in_maps = []
    for b in range(B):
        in_maps.append(
            {
                "xT": np.ascontiguousarray(qkv[b].T),
                "w_inT": w_inT,
                "bq8": bq8,
                "w_outT": w_outT,
                "bo": bo_eff,
            }
        )
    return in_maps


def kernel(qkv, w_in, b_in, w_out, b_out):
    from concourse.bass_utils import run_bass_kernel_spmd

    nc = build_nc()
    in_maps = prepare_in_maps(qkv, w_in, b_in, w_out, b_out)
    res = run_bass_kernel_spmd(nc, in_maps, core_ids=list(range(B)))
    out = np.stack(
        [np.ascontiguousarray(res.results[b]["outT"].T) for b in range(B)], axis=0
    )
    return out.astype(np.float32)


# revision 9
# speedup vs baseline: 1.1399x; 1.1399x over previous
"""Multi-head self-attention on 8 Trainium2 NeuronCores.

Sharding: data-parallel over batch (B=8 -> one batch element per core).
Each core runs the full MHA for its batch element; no collectives.

Per-core layout strategy (all matmuls in float32r = full-rate fp32):
  - host supplies xT = x.T [E,S] (e-major) and w_inT = w_in.T [E,3E].
  - in_proj Q/K computed transposed: projT[f,s] (f-major) -> per-head
    Q_hT/K_hT [d=64, s] slices are directly the matmul operands for
    scoresT[k,q] = K_hT.T @ Q_hT  (softmax scale 1/8 and q-bias folded
    into w_inT's Q columns / bq8 on host; k/v biases fold out of softmax
    or into the output bias).
  - in_proj V computed untransposed: proj_v[s, f] (s=k-major), stored
    head-strided with a ones column per head -> AV matmul
    lhsT=[V_h|1][k,65], rhs=attnT[k,q] yields out rows 0:64 = V^T E and
    row 64 = softmax denominator per q.
  - normalize: recip(denom row) -> partition_broadcast -> fused multiply
    on the PSUM->SBUF copyback into attnoutT[f,s].
  - out_proj transposed: finalT[e,s] = w_outT.T @ attnoutT (+ b_out_eff
    per-partition); host transposes back.
"""

import sys

if "/opt/trn_rl_repo" not in sys.path:
    sys.path.insert(0, "/opt/trn_rl_repo")

import numpy as np

B, S, E, H, D = 8, 1024, 1024, 16, 64
P = 128
EC = E // P  # 8 e-chunks
SB = S // 512  # 2 s-blocks

_NC = None
DEBUG_DUMPS = False
MM_BF16 = True  # matmul operands in bf16 (else float32r)


def attention_body(
    nc, tc, Act, f32, f32r, projq, projk, projv, attnout, attn_p, dnm_p, bca_p, ps_s, ps_av, dbg=None
):
    for qb in range(SB):
        for hp in range(EC):  # head pair = f-chunk of projq/projk
            at_tiles = [[None] * EC for _ in range(2)]
            for kc in range(EC):
                for half in range(2):
                    ps = ps_s.tile([P, 512], f32, name="ps_sc")
                    r0, r1 = half * 64, (half + 1) * 64
                    nc.tensor.matmul(
                        ps[:, :],
                        projk[hp][r0:r1, kc * 128 : (kc + 1) * 128],
                        projq[hp][r0:r1, qb * 512 : (qb + 1) * 512],
                        start=True,
                        stop=True,
                        tile_position=(64 * half, 0),
                    )
                    at = attn_p.tile([P, 512], f32r, name="at")
                    nc.scalar.activation(at[:, :], ps[:, :], Act.Exp)
                    at_tiles[half][kc] = at
                    if dbg is not None and DEBUG_DUMPS and qb == 0 and hp == 0 and kc == 0 and half == 0:
                        nc.sync.dma_start(out=dbg["at00"][:, :], in_=at[:, :])
            for half in range(2):
                h = hp * 2 + half
                pv = ps_av.tile([65, 512], f32, name="pv")
                for kc in range(EC):
                    nc.tensor.matmul(
                        pv[:, :],
                        projv[kc][:, h * 65 : h * 65 + 65],
                        at_tiles[half][kc][:, :],
                        start=(kc == 0),
                        stop=(kc == EC - 1),
                    )
                dr = dnm_p.tile([1, 512], f32, name="dr")
                nc.vector.tensor_copy(dr[:, :], pv[64:65, :])
                rc = dnm_p.tile([1, 512], f32, name="rc")
                nc.vector.reciprocal_approx_fast(rc[:, :], dr[:, :])
                bc = bca_p.tile([64, 512], f32, name="bc")
                nc.gpsimd.partition_broadcast(bc[:, :], rc[:, :])
                if dbg is not None and DEBUG_DUMPS and qb == 0 and hp == 0 and half == 0:
                    nc.sync.dma_start(out=dbg["bc0"][:, :], in_=bc[:, :])
                nc.vector.tensor_mul(
                    attnout[hp][
                        half * 64 : (half + 1) * 64,
                        qb * 512 : (qb + 1) * 512,
                    ],
                    pv[0:64, :],
                    bc[:, :],
                )


def build_nc():
    global _NC
    if _NC is not None:
        return _NC

    import concourse.mybir as mybir
    from concourse import bacc
    from concourse.tile import TileContext

    f32 = mybir.dt.float32
    f32r = mybir.dt.bfloat16 if MM_BF16 else mybir.dt.float32r
    Act = mybir.ActivationFunctionType

    nc = bacc.Bacc("TRN2", target_bir_lowering=False, debug=False, num_devices=8)

    xT_d = nc.dram_tensor("xT", [E, S], f32r, kind="ExternalInput").ap()
    w_inT_d = nc.dram_tensor("w_inT", [E, 3 * E], f32r, kind="ExternalInput").ap()
    bq8_d = nc.dram_tensor("bq8", [E, 1], f32, kind="ExternalInput").ap()
    w_outT_d = nc.dram_tensor("w_outT", [E, E], f32r, kind="ExternalInput").ap()
    bo_d = nc.dram_tensor("bo", [E, 1], f32, kind="ExternalInput").ap()
    outT_d = nc.dram_tensor("outT", [E, S], f32, kind="ExternalOutput").ap()
    dbg = {}
    if DEBUG_DUMPS:
        dbg["projq0"] = nc.dram_tensor("d_projq0", [P, S], f32r, kind="ExternalOutput").ap()
        dbg["projk0"] = nc.dram_tensor("d_projk0", [P, S], f32r, kind="ExternalOutput").ap()
        dbg["projv0"] = nc.dram_tensor("d_projv0", [P, H * 65], f32r, kind="ExternalOutput").ap()
        dbg["at00"] = nc.dram_tensor("d_at00", [P, 512], f32r, kind="ExternalOutput").ap()
        dbg["attnout0"] = nc.dram_tensor("d_attnout0", [P, S], f32r, kind="ExternalOutput").ap()
        dbg["bc0"] = nc.dram_tensor("d_bc0", [64, 512], f32, kind="ExternalOutput").ap()

    with TileContext(nc) as tc:
        with (
            tc.tile_pool(name="const", bufs=1) as const,
            tc.tile_pool(name="projq", bufs=1) as projq_p,
            tc.tile_pool(name="projk", bufs=1) as projk_p,
            tc.tile_pool(name="projv", bufs=1) as projv_p,
        ):
            bq8_t = []
            bo_t = []
            for i in range(EC):
                t = const.tile([P, 1], f32, tag=f"bq{i}", name=f"bq{i}")
                nc.sync.dma_start(out=t[:, :], in_=bq8_d[i * P : (i + 1) * P, :])
                bq8_t.append(t)
                t = const.tile([P, 1], f32, tag=f"bo{i}", name=f"bo{i}")
                nc.sync.dma_start(out=t[:, :], in_=bo_d[i * P : (i + 1) * P, :])
                bo_t.append(t)

            projq = [projq_p.tile([P, S], f32r, tag=f"pq{i}", name=f"pq{i}") for i in range(EC)]
            projk = [projk_p.tile([P, S], f32r, tag=f"pk{i}", name=f"pk{i}") for i in range(EC)]
            # per s-chunk: 16 heads x (64 V cols + ones col)
            projv = [projv_p.tile([P, H * 65], f32r, tag=f"pv{i}", name=f"pv{i}") for i in range(EC)]

            # ---------------- Phase A: in_proj ----------------
            with (
                tc.tile_pool(name="xT", bufs=1) as xT_p,
                tc.tile_pool(name="w_in", bufs=16) as w_in_p,
                tc.tile_pool(name="ps_a", bufs=6, space="PSUM") as ps_a,
            ):
                xT = []
                for e in range(EC):
                    t = xT_p.tile([P, S], f32r, tag=f"x{e}", name=f"x{e}")
                    nc.sync.dma_start(out=t[:, :], in_=xT_d[e * P : (e + 1) * P, :])
                    xT.append(t)

                # ones columns of projv (col 64 of each head's 65-col group)
                ones_f32 = const.tile([P, H], f32, name="ones_f32")
                nc.vector.memset(ones_f32[:, :], 1.0)
                ones_src = ones_f32[:, :].rearrange("p (h o) -> p h o", o=1)
                for sc in range(EC):
                    ones_ap = projv[sc].rearrange("p (h x) -> p h x", x=65)[:, :, 64:65]
                    nc.vector.tensor_copy(ones_ap, ones_src)

                for fg in range(6):  # f-groups of 512 cols: 0,1=Q 2,3=K 4,5=V
                    w_t = []
                    for e in range(EC):
                        t = w_in_p.tile([P, 512], f32r, name="w_t")
                        nc.sync.dma_start(
                            out=t[:, :],
                            in_=w_inT_d[e * P : (e + 1) * P, fg * 512 : (fg + 1) * 512],
                        )
                        w_t.append(t)
                    if fg < 4:  # Q/K: projT[f,s] orientation
                        for fc in range(4):
                            fglob = fg * 4 + fc  # global f-chunk 0..15
                            for sb in range(SB):
                                ps = ps_a.tile([P, 512], f32)
                                for e in range(EC):
                                    nc.tensor.matmul(
                                        ps[:, :],
                                        w_t[e][:, fc * 128 : (fc + 1) * 128],
                                        xT[e][:, sb * 512 : (sb + 1) * 512],
                                        start=(e == 0),
                                        stop=(e == EC - 1),
                                    )
                                if fg < 2:  # Q: add folded bias
                                    nc.vector.tensor_scalar_add(
                                        projq[fglob][:, sb * 512 : (sb + 1) * 512],
                                        ps[:, :],
                                        bq8_t[fglob][:, 0:1],
                                    )
                                else:  # K: plain copy
                                    nc.vector.tensor_copy(
                                        projk[fglob - 8][:, sb * 512 : (sb + 1) * 512],
                                        ps[:, :],
                                    )
                    else:  # V: proj_v[s, f] orientation
                        h0 = (fg - 4) * 8  # first head in this 512-col group
                        for sc in range(EC):
                            ps = ps_a.tile([P, 512], f32)
                            for e in range(EC):
                                nc.tensor.matmul(
                                    ps[:, :],
                                    xT[e][:, sc * 128 : (sc + 1) * 128],
                                    w_t[e][:, :],
                                    start=(e == 0),
                                    stop=(e == EC - 1),
                                )
                            out_ap = projv[sc].rearrange("p (h x) -> p h x", x=65)[
                                :, h0 : h0 + 8, 0:64
                            ]
                            in_ap = ps.rearrange("p (h d) -> p h d", d=64)
                            nc.vector.tensor_copy(out_ap, in_ap)

            if DEBUG_DUMPS:
                nc.sync.dma_start(out=dbg["projq0"][:, :], in_=projq[0][:, :])
                nc.sync.dma_start(out=dbg["projk0"][:, :], in_=projk[0][:, :])
                nc.sync.dma_start(out=dbg["projv0"][:, :], in_=projv[0][:, :])

            # ---------------- Phase B: attention ----------------
            with tc.tile_pool(name="attnout", bufs=1) as attnout_p:
                attnout = [
                    attnout_p.tile([P, S], f32r, tag=f"ao{i}", name=f"ao{i}") for i in range(EC)
                ]
                with (
                    tc.tile_pool(name="attn", bufs=18) as attn_p,
                    tc.tile_pool(name="dnm", bufs=4) as dnm_p,
                    tc.tile_pool(name="bca", bufs=4) as bca_p,
                    tc.tile_pool(name="ps_s", bufs=6, space="PSUM") as ps_s,
                    tc.tile_pool(name="ps_av", bufs=2, space="PSUM") as ps_av,
                ):
                    attention_body(nc, tc, Act, f32, f32r, projq, projk, projv, attnout, attn_p, dnm_p, bca_p, ps_s, ps_av, dbg)

                if DEBUG_DUMPS:
                    nc.sync.dma_start(out=dbg["attnout0"][:, :], in_=attnout[0][:, :])

                # ---------------- Phase C: out_proj ----------------
                with (
                    tc.tile_pool(name="w_out", bufs=1) as w_out_p,
                    tc.tile_pool(name="fin", bufs=4) as fin_p,
                    tc.tile_pool(name="ps_o", bufs=4, space="PSUM") as ps_o,
                ):
                    w_out_t = []
                    for fc in range(EC):
                        t = w_out_p.tile([P, E], f32r, tag=f"wo{fc}", name=f"wo{fc}")
                        nc.sync.dma_start(
                            out=t[:, :], in_=w_outT_d[fc * P : (fc + 1) * P, :]
                        )
                        w_out_t.append(t)
                    for qb in range(SB):
                        for ec in range(EC):
                            ps = ps_o.tile([P, 512], f32)
                            for fc in range(EC):
                                nc.tensor.matmul(
                                    ps[:, :],
                                    w_out_t[fc][:, ec * 128 : (ec + 1) * 128],
                                    attnout[fc][:, qb * 512 : (qb + 1) * 512],
                                    start=(fc == 0),
                                    stop=(fc == EC - 1),
                                )
                            fin = fin_p.tile([P, 512], f32)
                            nc.vector.tensor_scalar_add(
                                fin[:, :], ps[:, :], bo_t[ec][:, 0:1]
                            )
                            nc.sync.dma_start(
                                out=outT_d[
                                    ec * P : (ec + 1) * P, qb * 512 : (qb + 1) * 512
                                ],
                                in_=fin[:, :],
                            )

    nc.compile()
    _NC = nc
    return nc


def prepare_in_maps(qkv, w_in, b_in, w_out, b_out):
    qkv = np.asarray(qkv, dtype=np.float32)
    w_in = np.asarray(w_in, dtype=np.float32)
    b_in = np.asarray(b_in, dtype=np.float32)
    w_out = np.asarray(w_out, dtype=np.float32)
    b_out = np.asarray(b_out, dtype=np.float32)

    w_inT = np.ascontiguousarray(w_in.T).copy()
    w_inT[:, :E] *= 0.125  # fold softmax scale into Q columns
    bq8 = (b_in[:E] * 0.125).astype(np.float32).reshape(E, 1)
    # v-bias folds through attention (rows of softmax sum to 1) into out bias
    bo_eff = (b_out + w_out @ b_in[2 * E :]).astype(np.float32).reshape(E, 1)
    w_outT = np.ascontiguousarray(w_out.T)

    if MM_BF16:
        import ml_dtypes

        bf = ml_dtypes.bfloat16
        w_inT = w_inT.astype(bf)
        w_outT = w_outT.astype(bf)
    in_maps = []
    for b in range(B):
        xTb = np.ascontiguousarray(qkv[b].T)
        if MM_BF16:
            xTb = xTb.astype(bf)
        in_maps.append(
            {
                "xT": xTb,
                "w_inT": w_inT,
                "bq8": bq8,
                "w_outT": w_outT,
                "bo": bo_eff,
            }
        )
    return in_maps


def kernel(qkv, w_in, b_in, w_out, b_out):
    from concourse.bass_utils import run_bass_kernel_spmd

    nc = build_nc()
    in_maps = prepare_in_maps(qkv, w_in, b_in, w_out, b_out)
    res = run_bass_kernel_spmd(nc, in_maps, core_ids=list(range(B)))
    out = np.stack(
        [np.ascontiguousarray(res.results[b]["outT"].T) for b in range(B)], axis=0
    )
    return out.astype(np.float32)


# revision 10
# speedup vs baseline: 1.5232x; 1.3362x over previous
"""Multi-head self-attention on 8 Trainium2 NeuronCores.

Sharding: data-parallel over batch (B=8 -> one batch element per core).
Each core runs the full MHA for its batch element; no collectives.

Per-core layout strategy (all matmuls in float32r = full-rate fp32):
  - host supplies xT = x.T [E,S] (e-major) and w_inT = w_in.T [E,3E].
  - in_proj Q/K computed transposed: projT[f,s] (f-major) -> per-head
    Q_hT/K_hT [d=64, s] slices are directly the matmul operands for
    scoresT[k,q] = K_hT.T @ Q_hT  (softmax scale 1/8 and q-bias folded
    into w_inT's Q columns / bq8 on host; k/v biases fold out of softmax
    or into the output bias).
  - in_proj V computed untransposed: proj_v[s, f] (s=k-major), stored
    head-strided with a ones column per head -> AV matmul
    lhsT=[V_h|1][k,65], rhs=attnT[k,q] yields out rows 0:64 = V^T E and
    row 64 = softmax denominator per q.
  - normalize: recip(denom row) -> partition_broadcast -> fused multiply
    on the PSUM->SBUF copyback into attnoutT[f,s].
  - out_proj transposed: finalT[e,s] = w_outT.T @ attnoutT (+ b_out_eff
    per-partition); host transposes back.
"""

import sys

if "/opt/trn_rl_repo" not in sys.path:
    sys.path.insert(0, "/opt/trn_rl_repo")

import numpy as np

B, S, E, H, D = 8, 1024, 1024, 16, 64
P = 128
EC = E // P  # 8 e-chunks
SB = S // 512  # 2 s-blocks

_NC = None
DEBUG_DUMPS = False
MM_BF16 = True  # matmul operands in bf16 (else float32r)


def attention_body(
    nc, tc, Act, f32, f32r, projq, projk, projv, attnout, attn_p, dnm_p, bca_p, ps_s, ps_av, dbg=None
):
    for qb in range(SB):
        for hp in range(EC):  # head pair = f-chunk of projq/projk
            at_tiles = [[None] * EC for _ in range(2)]
            for kc in range(EC):
                # both heads of the pair -> two halves (two banks) of one psum
                # tile, MMs emitted back-to-back so the row-group packing engages
                ps = ps_s.tile([P, 1024], f32, name="ps_sc")
                for half in range(2):
                    r0, r1 = half * 64, (half + 1) * 64
                    nc.tensor.matmul(
                        ps[:, half * 512 : (half + 1) * 512],
                        projk[hp][r0:r1, kc * 128 : (kc + 1) * 128],
                        projq[hp][r0:r1, qb * 512 : (qb + 1) * 512],
                        start=True,
                        stop=True,
                        tile_position=(64 * half, 0),
                    )
                at = attn_p.tile([P, 1024], f32r, name="at")
                nc.scalar.activation(at[:, :], ps[:, :], Act.Exp)
                for half in range(2):
                    at_tiles[half][kc] = at[:, half * 512 : (half + 1) * 512]
                if dbg is not None and DEBUG_DUMPS and qb == 0 and hp == 0 and kc == 0:
                    nc.sync.dma_start(out=dbg["at00"][:, :], in_=at[:, 0:512])
            for half in range(2):
                h = hp * 2 + half
                pv = ps_av.tile([65, 512], f32, name="pv")
                for kc in range(EC):
                    nc.tensor.matmul(
                        pv[:, :],
                        projv[kc][:, h * 65 : h * 65 + 65],
                        at_tiles[half][kc],
                        start=(kc == 0),
                        stop=(kc == EC - 1),
                    )
                dr = dnm_p.tile([1, 512], f32, name="dr")
                nc.vector.tensor_copy(dr[:, :], pv[64:65, :])
                rc = dnm_p.tile([1, 512], f32, name="rc")
                nc.vector.reciprocal_approx_fast(rc[:, :], dr[:, :])
                bc = bca_p.tile([64, 512], f32, name="bc")
                nc.gpsimd.partition_broadcast(bc[:, :], rc[:, :])
                if dbg is not None and DEBUG_DUMPS and qb == 0 and hp == 0 and half == 0:
                    nc.sync.dma_start(out=dbg["bc0"][:, :], in_=bc[:, :])
                nc.vector.tensor_mul(
                    attnout[hp][
                        half * 64 : (half + 1) * 64,
                        qb * 512 : (qb + 1) * 512,
                    ],
                    pv[0:64, :],
                    bc[:, :],
                )


def build_nc():
    global _NC
    if _NC is not None:
        return _NC

    import concourse.mybir as mybir
    from concourse import bacc
    from concourse.tile import TileContext

    f32 = mybir.dt.float32
    f32r = mybir.dt.bfloat16 if MM_BF16 else mybir.dt.float32r
    Act = mybir.ActivationFunctionType

    nc = bacc.Bacc("TRN2", target_bir_lowering=False, debug=False, num_devices=8)

    xT_d = nc.dram_tensor("xT", [E, S], f32r, kind="ExternalInput").ap()
    w_inT_d = nc.dram_tensor("w_inT", [E, 3 * E], f32r, kind="ExternalInput").ap()
    bq8_d = nc.dram_tensor("bq8", [E, 1], f32, kind="ExternalInput").ap()
    w_outT_d = nc.dram_tensor("w_outT", [E, E], f32r, kind="ExternalInput").ap()
    bo_d = nc.dram_tensor("bo", [E, 1], f32, kind="ExternalInput").ap()
    outT_d = nc.dram_tensor("outT", [E, S], f32, kind="ExternalOutput").ap()
    dbg = {}
    if DEBUG_DUMPS:
        dbg["projq0"] = nc.dram_tensor("d_projq0", [P, S], f32r, kind="ExternalOutput").ap()
        dbg["projk0"] = nc.dram_tensor("d_projk0", [P, S], f32r, kind="ExternalOutput").ap()
        dbg["projv0"] = nc.dram_tensor("d_projv0", [P, H * 65], f32r, kind="ExternalOutput").ap()
        dbg["at00"] = nc.dram_tensor("d_at00", [P, 512], f32r, kind="ExternalOutput").ap()
        dbg["attnout0"] = nc.dram_tensor("d_attnout0", [P, S], f32r, kind="ExternalOutput").ap()
        dbg["bc0"] = nc.dram_tensor("d_bc0", [64, 512], f32, kind="ExternalOutput").ap()

    with TileContext(nc) as tc:
        with (
            tc.tile_pool(name="const", bufs=1) as const,
            tc.tile_pool(name="projq", bufs=1) as projq_p,
            tc.tile_pool(name="projk", bufs=1) as projk_p,
            tc.tile_pool(name="projv", bufs=1) as projv_p,
        ):
            bq8_t = []
            bo_t = []
            for i in range(EC):
                t = const.tile([P, 1], f32, tag=f"bq{i}", name=f"bq{i}")
                nc.sync.dma_start(out=t[:, :], in_=bq8_d[i * P : (i + 1) * P, :])
                bq8_t.append(t)
                t = const.tile([P, 1], f32, tag=f"bo{i}", name=f"bo{i}")
                nc.sync.dma_start(out=t[:, :], in_=bo_d[i * P : (i + 1) * P, :])
                bo_t.append(t)

            projq = [projq_p.tile([P, S], f32r, tag=f"pq{i}", name=f"pq{i}") for i in range(EC)]
            projk = [projk_p.tile([P, S], f32r, tag=f"pk{i}", name=f"pk{i}") for i in range(EC)]
            # per s-chunk: 16 heads x (64 V cols + ones col)
            projv = [projv_p.tile([P, H * 65], f32r, tag=f"pv{i}", name=f"pv{i}") for i in range(EC)]

            # ---------------- Phase A: in_proj ----------------
            with (
                tc.tile_pool(name="xT", bufs=1) as xT_p,
                tc.tile_pool(name="w_in", bufs=16) as w_in_p,
                tc.tile_pool(name="ps_a", bufs=6, space="PSUM") as ps_a,
            ):
                xT = []
                for e in range(EC):
                    t = xT_p.tile([P, S], f32r, tag=f"x{e}", name=f"x{e}")
                    nc.sync.dma_start(out=t[:, :], in_=xT_d[e * P : (e + 1) * P, :])
                    xT.append(t)

                # ones columns of projv (col 64 of each head's 65-col group)
                ones_f32 = const.tile([P, H], f32, name="ones_f32")
                nc.vector.memset(ones_f32[:, :], 1.0)
                ones_src = ones_f32[:, :].rearrange("p (h o) -> p h o", o=1)
                for sc in range(EC):
                    ones_ap = projv[sc].rearrange("p (h x) -> p h x", x=65)[:, :, 64:65]
                    nc.vector.tensor_copy(ones_ap, ones_src)

                for fg in [4, 5, 0, 2, 1, 3]:  # V first so attention can start early
                    w_t = []
                    for e in range(EC):
                        t = w_in_p.tile([P, 512], f32r, name="w_t")
                        nc.sync.dma_start(
                            out=t[:, :],
                            in_=w_inT_d[e * P : (e + 1) * P, fg * 512 : (fg + 1) * 512],
                        )
                        w_t.append(t)
                    if fg < 4:  # Q/K: projT[f,s] orientation
                        for fc in range(4):
                            fglob = fg * 4 + fc  # global f-chunk 0..15
                            for sb in range(SB):
                                ps = ps_a.tile([P, 512], f32)
                                for e in range(EC):
                                    nc.tensor.matmul(
                                        ps[:, :],
                                        w_t[e][:, fc * 128 : (fc + 1) * 128],
                                        xT[e][:, sb * 512 : (sb + 1) * 512],
                                        start=(e == 0),
                                        stop=(e == EC - 1),
                                    )
                                if fg < 2:  # Q: add folded bias
                                    nc.vector.tensor_scalar_add(
                                        projq[fglob][:, sb * 512 : (sb + 1) * 512],
                                        ps[:, :],
                                        bq8_t[fglob][:, 0:1],
                                    )
                                else:  # K: plain copy
                                    nc.vector.tensor_copy(
                                        projk[fglob - 8][:, sb * 512 : (sb + 1) * 512],
                                        ps[:, :],
                                    )
                    else:  # V: proj_v[s, f] orientation
                        h0 = (fg - 4) * 8  # first head in this 512-col group
                        for sc in range(EC):
                            ps = ps_a.tile([P, 512], f32)
                            for e in range(EC):
                                nc.tensor.matmul(
                                    ps[:, :],
                                    xT[e][:, sc * 128 : (sc + 1) * 128],
                                    w_t[e][:, :],
                                    start=(e == 0),
                                    stop=(e == EC - 1),
                                )
                            out_ap = projv[sc].rearrange("p (h x) -> p h x", x=65)[
                                :, h0 : h0 + 8, 0:64
                            ]
                            in_ap = ps.rearrange("p (h d) -> p h d", d=64)
                            nc.vector.tensor_copy(out_ap, in_ap)

            if DEBUG_DUMPS:
                nc.sync.dma_start(out=dbg["projq0"][:, :], in_=projq[0][:, :])
                nc.sync.dma_start(out=dbg["projk0"][:, :], in_=projk[0][:, :])
                nc.sync.dma_start(out=dbg["projv0"][:, :], in_=projv[0][:, :])

            # ---------------- Phase B: attention ----------------
            with tc.tile_pool(name="attnout", bufs=1) as attnout_p:
                attnout = [
                    attnout_p.tile([P, S], f32r, tag=f"ao{i}", name=f"ao{i}") for i in range(EC)
                ]
                with (
                    tc.tile_pool(name="attn", bufs=9) as attn_p,
                    tc.tile_pool(name="dnm", bufs=4) as dnm_p,
                    tc.tile_pool(name="bca", bufs=4) as bca_p,
                    tc.tile_pool(name="ps_s", bufs=3, space="PSUM") as ps_s,
                    tc.tile_pool(name="ps_av", bufs=2, space="PSUM") as ps_av,
                ):
                    attention_body(nc, tc, Act, f32, f32r, projq, projk, projv, attnout, attn_p, dnm_p, bca_p, ps_s, ps_av, dbg)

                if DEBUG_DUMPS:
                    nc.sync.dma_start(out=dbg["attnout0"][:, :], in_=attnout[0][:, :])

                # ---------------- Phase C: out_proj ----------------
                with (
                    tc.tile_pool(name="w_out", bufs=1) as w_out_p,
                    tc.tile_pool(name="fin", bufs=4) as fin_p,
                    tc.tile_pool(name="ps_o", bufs=4, space="PSUM") as ps_o,
                ):
                    w_out_t = []
                    for fc in range(EC):
                        t = w_out_p.tile([P, E], f32r, tag=f"wo{fc}", name=f"wo{fc}")
                        nc.sync.dma_start(
                            out=t[:, :], in_=w_outT_d[fc * P : (fc + 1) * P, :]
                        )
                        w_out_t.append(t)
                    for qb in range(SB):
                        for ec in range(EC):
                            ps = ps_o.tile([P, 512], f32)
                            for fc in range(EC):
                                nc.tensor.matmul(
                                    ps[:, :],
                                    w_out_t[fc][:, ec * 128 : (ec + 1) * 128],
                                    attnout[fc][:, qb * 512 : (qb + 1) * 512],
                                    start=(fc == 0),
                                    stop=(fc == EC - 1),
                                )
                            fin = fin_p.tile([P, 512], f32)
                            nc.vector.tensor_scalar_add(
                                fin[:, :], ps[:, :], bo_t[ec][:, 0:1]
                            )
                            nc.sync.dma_start(
                                out=outT_d[
                                    ec * P : (ec + 1) * P, qb * 512 : (qb + 1) * 512
                                ],
                                in_=fin[:, :],
                            )

    nc.compile()
    _NC = nc
    return nc


def prepare_in_maps(qkv, w_in, b_in, w_out, b_out):
    qkv = np.asarray(qkv, dtype=np.float32)
    w_in = np.asarray(w_in, dtype=np.float32)
    b_in = np.asarray(b_in, dtype=np.float32)
    w_out = np.asarray(w_out, dtype=np.float32)
    b_out = np.asarray(b_out, dtype=np.float32)

    w_inT = np.ascontiguousarray(w_in.T).copy()
    w_inT[:, :E] *= 0.125  # fold softmax scale into Q columns
    bq8 = (b_in[:E] * 0.125).astype(np.float32).reshape(E, 1)
    # v-bias folds through attention (rows of softmax sum to 1) into out bias
    bo_eff = (b_out + w_out @ b_in[2 * E :]).astype(np.float32).reshape(E, 1)
    w_outT = np.ascontiguousarray(w_out.T)

    if MM_BF16:
        import ml_dtypes

        bf = ml_dtypes.bfloat16
        w_inT = w_inT.astype(bf)
        w_outT = w_outT.astype(bf)
    in_maps = []
    for b in range(B):
        xTb = np.ascontiguousarray(qkv[b].T)
        if MM_BF16:
            xTb = xTb.astype(bf)
        in_maps.append(
            {
                "xT": xTb,
                "w_inT": w_inT,
                "bq8": bq8,
                "w_outT": w_outT,
                "bo": bo_eff,
            }
        )
    return in_maps


def kernel(qkv, w_in, b_in, w_out, b_out):
    from concourse.bass_utils import run_bass_kernel_spmd

    nc = build_nc()
    in_maps = prepare_in_maps(qkv, w_in, b_in, w_out, b_out)
    res = run_bass_kernel_spmd(nc, in_maps, core_ids=list(range(B)))
    out = np.stack(
        [np.ascontiguousarray(res.results[b]["outT"].T) for b in range(B)], axis=0
    )
    return out.astype(np.float32)
